# revision 1
# baseline (speedup 1.0000x reference)
"""AttentionMemoryInterface Trainium2 kernel (v3).

Reference computation per batch element b (memory [N=4096, D=128], x [256]):
    mv = x@W_write+b_write; wq = x@W_wq+b_wq; rq = x@W_rq+b_rq
    wl[n] = mem[n,:]@wq ; ww = softmax(wl)
    new_mem = mem*(1-ww) + mv*ww
    rl[n] = new_mem[n,:]@rq ; rw = softmax(rl)
    out = (rw @ new_mem) @ W_ro + b_ro

Algebraic restructure (new_mem never materialized):
    lr[n] = mem[n,:]@rq                  (same pass as wl)
    cbar  = mv@rq                        (scalar per b)
    rl[n] = lr[n] + ww[n]*(cbar - lr[n])
    g[n]  = rw[n]*(1-ww[n]);  s = sum_n rw[n]*ww[n]
    read_out = sum_n g[n]*mem[n,:] + s*mv
    out = read_out @ W_ro + b_ro

v3 structure (per core, 8 batch elements, data-parallel over batch):
  - host pre-transposes memory -> memT [8, 128(d), 4096(n)]; DMA'd flat
    (perfect per-partition-contiguous pattern); no on-chip memory transposes.
  - pass 1: col-tiled matmuls - stationary QP32 [128,32] at 4 column-groups
    of the PE array, moving memT groups [128,512]; 4 concurrent streams fill
    one PSUM bank [128,512] (partitions 32j+q).
  - logit fix-up: 4 PE transposes per bank -> [128(n-sub), (Q,j,q)] layout so
    softmax runs on 128-lane tiles.
  - softmax: ACT exp (with accumulated row sums), DVE elementwise, PE for
    cross-partition sums/broadcasts.
  - g row: SWDGE fold-DMA [128,32] -> [1,4096] in n-order (optionally cast
    f32r); broadcast to [128,512] PSUM quarters via K=1 PE matmuls.
  - pass 2: DVE scalar_tensor_tensor (memT-quarter * g-bcast) with accum_out
    per-partition sums -> read_out; epilogue matmul with bias folded in.
"""

import numpy as np

import concourse.bass as bass
import concourse.bacc as bacc
import concourse.mybir as mybir
import concourse.tile as tile
from concourse.bass_utils import run_bass_kernel_spmd

N_CORES = 8
B, IN_DIM, D, N_SLOTS = 64, 256, 128, 4096
BC = B // N_CORES          # batch per core
NQ = 2 * BC                # 16 query columns (wq x 8 | rq x 8)
NGRP = N_SLOTS // 512      # 8 moving groups per b
F32 = mybir.dt.float32
F32R = mybir.dt.float32r
AX = mybir.AxisListType
ALU = mybir.AluOpType
ACTF = mybir.ActivationFunctionType


def build_nc(loop_n: int = 1, phase: str = "full", bcast: str = "pe_f32r"):
    nc = bacc.Bacc("TRN2", target_bir_lowering=False, debug=False,
                   num_devices=N_CORES)

    x_d = nc.dram_tensor("x", [BC, IN_DIM], F32, kind="ExternalInput")
    memt_d = nc.dram_tensor("memoryT", [BC, D, N_SLOTS], F32,
                            kind="ExternalInput")
    w_wr_d = nc.dram_tensor("W_write", [IN_DIM, D], F32, kind="ExternalInput")
    b_wr_d = nc.dram_tensor("b_write", [1, D], F32, kind="ExternalInput")
    w_wq_d = nc.dram_tensor("W_wq", [IN_DIM, D], F32, kind="ExternalInput")
    b_wq_d = nc.dram_tensor("b_wq", [1, D], F32, kind="ExternalInput")
    w_rq_d = nc.dram_tensor("W_rq", [IN_DIM, D], F32, kind="ExternalInput")
    b_rq_d = nc.dram_tensor("b_rq", [1, D], F32, kind="ExternalInput")
    w_ro_d = nc.dram_tensor("W_ro", [D, IN_DIM], F32, kind="ExternalInput")
    b_ro_d = nc.dram_tensor("b_ro", [1, IN_DIM], F32, kind="ExternalInput")
    ident_d = nc.dram_tensor("ident", [128, 128], F32, kind="ExternalInput")
    onesc_d = nc.dram_tensor("ones_col", [128, 1], F32, kind="ExternalInput")
    onesr_d = nc.dram_tensor("ones_row", [1, 128], F32, kind="ExternalInput")
    out_d = nc.dram_tensor("out", [BC, IN_DIM], F32, kind="ExternalOutput")

    kw = dict(x=x_d.ap(), memt=memt_d.ap(),
              w_wr=w_wr_d.ap(), b_wr=b_wr_d.ap(),
              w_wq=w_wq_d.ap(), b_wq=b_wq_d.ap(),
              w_rq=w_rq_d.ap(), b_rq=b_rq_d.ap(),
              w_ro=w_ro_d.ap(), b_ro=b_ro_d.ap(),
              ident=ident_d.ap(), ones_col=onesc_d.ap(),
              ones_row=onesr_d.ap(), out=out_d.ap(),
              phase=phase, bcast=bcast)
    with tile.TileContext(nc) as tc:
        if loop_n == 1:
            _body(nc, tc, **kw)
        else:
            with tc.For_i(0, loop_n, 1):
                _body(nc, tc, **kw)
    nc.compile()
    return nc


def _body(nc, tc, *, x, memt, w_wr, b_wr, w_wq, b_wq, w_rq, b_rq,
          w_ro, b_ro, ident, ones_col, ones_row, out, phase, bcast):
    from contextlib import ExitStack
    ctx = ExitStack()
    gdt = F32R if bcast == "pe_f32r" else F32
    with ctx:
        consts = ctx.enter_context(tc.tile_pool(name="consts", bufs=1))
        mtp = ctx.enter_context(tc.tile_pool(name="mt", bufs=1))
        ctp = ctx.enter_context(tc.tile_pool(name="ct", bufs=3))
        ltp = ctx.enter_context(tc.tile_pool(name="lt", bufs=3))
        sm = ctx.enter_context(tc.tile_pool(name="sm", bufs=2))
        grp = ctx.enter_context(tc.tile_pool(name="gr", bufs=2))
        ps_ct = ctx.enter_context(tc.tile_pool(name="ps_ct", bufs=2, space="PSUM"))
        ps_fx = ctx.enter_context(tc.tile_pool(name="ps_fx", bufs=1, space="PSUM"))
        ps_gb = ctx.enter_context(tc.tile_pool(name="ps_gb", bufs=2, space="PSUM"))
        ps_sm = ctx.enter_context(tc.tile_pool(name="ps_sm", bufs=2, space="PSUM"))

        # ---------- constants ----------
        ident_sb = consts.tile([128, 128], F32, tag="ident", name="ident_sb")
        nc.sync.dma_start(ident_sb[:], ident)
        onesc_sb = consts.tile([128, 1], F32, tag="onesc", name="onesc_sb")
        nc.sync.dma_start(onesc_sb[:], ones_col)
        onesr_sb = consts.tile([1, 128], F32, tag="onesr", name="onesr_sb")
        nc.sync.dma_start(onesr_sb[:], ones_row)
        onesr_r = consts.tile([1, 128], gdt, tag="onesr_r", name="onesr_r")
        nc.gpsimd.dma_start(onesr_r[:], ones_row)

        w_ro_sb = consts.tile([D, IN_DIM], F32, tag="wro", name="w_ro_sb")
        nc.sync.dma_start(w_ro_sb[:], w_ro)
        b_ro_sb = consts.tile([1, IN_DIM], F32, tag="bro", name="b_ro_sb")
        nc.sync.dma_start(b_ro_sb[:], b_ro)

        proj_w = []
        for name, wd, bd in (("wr", w_wr, b_wr), ("wq", w_wq, b_wq),
                             ("rq", w_rq, b_rq)):
            chunks = []
            for k in range(IN_DIM // 128):
                wt = consts.tile([128, D], F32, tag=f"w_{name}{k}",
                                 name=f"w_{name}{k}")
                nc.sync.dma_start(wt[:], wd[k * 128:(k + 1) * 128, :])
                chunks.append(wt)
            bt = consts.tile([1, D], F32, tag=f"b_{name}", name=f"b_{name}")
            nc.sync.dma_start(bt[:], bd)
            proj_w.append((chunks, bt))

        x_nat = consts.tile([BC, IN_DIM], F32, tag="xnat", name="x_nat")
        nc.sync.dma_start(x_nat[:], x)

        # ---------- memory DMAs (flat, per-partition contiguous) ----------
        m_tiles = []
        for b in range(BC):
            mb = mtp.tile([128, N_SLOTS], F32, tag=f"mem{b}", name=f"memt{b}")
            nc.sync.dma_start(mb[:], memt[b])
            m_tiles.append(mb)

        if phase == "dma":
            dummy = sm.tile([128, BC], F32, tag="dummy", name="dummy")
            for b in range(BC):
                nc.vector.tensor_copy(dummy[:, b:b + 1], m_tiles[b][:, 0:1])
            return

        # ---------- x transpose ----------
        xt = []
        for k in range(IN_DIM // 128):
            ps = ps_sm.tile([128, BC], F32, tag="ps_small", name=f"ps_xt{k}")
            nc.tensor.matmul(ps[:], x_nat[:, k * 128:(k + 1) * 128],
                             ident_sb[0:BC, 0:BC], is_transpose=True)
            t = consts.tile([128, BC], F32, tag=f"xt{k}", name=f"xt{k}")
            nc.scalar.activation(t[:], ps[:], ACTF.Copy)
            xt.append(t)

        # ---------- projections -> mv_t [128, BC], qp32 [128, 32] ----------
        # qp32 columns: [wq (8) | rq (8) | wq (8) | rq (8)] (duplicated to
        # fill all 32 stationary columns of each PE column-group).
        mv_t = consts.tile([128, BC], F32, tag="mvt", name="mv_t")
        qp32 = consts.tile([128, 32], F32, tag="qp32", name="qp32")
        for j, (chunks, bt) in enumerate(proj_w):
            ps = ps_sm.tile([128, BC], F32, tag="ps_small", name=f"ps_proj{j}")
            nc.tensor.matmul(ps[:], bt[:], onesr_sb[:, 0:BC], start=True,
                             stop=False)
            for k in range(IN_DIM // 128):
                nc.tensor.matmul(ps[:], chunks[k][:], xt[k][:],
                                 start=False, stop=(k == IN_DIM // 128 - 1))
            if j == 0:
                nc.scalar.activation(mv_t[:], ps[:], ACTF.Copy)
            else:
                off = (j - 1) * BC
                nc.scalar.activation(qp32[:, off:off + BC], ps[:], ACTF.Copy)
                nc.scalar.activation(qp32[:, 16 + off:16 + off + BC], ps[:],
                                     ACTF.Copy)

        # ---------- cbar ----------
        tmv = sm.tile([128, BC], F32, tag="tmv", name="tmv")
        nc.vector.tensor_tensor(tmv[:], mv_t[:], qp32[:, BC:2 * BC], ALU.mult)
        ps_c = ps_sm.tile([1, BC], F32, tag="ps_small", name="ps_crow")
        nc.tensor.matmul(ps_c[:], onesc_sb[:], tmv[:])
        c_row = consts.tile([1, BC], F32, tag="crow", name="c_row")
        nc.scalar.activation(c_row[:], ps_c[:], ACTF.Copy)
        ps_cb = ps_sm.tile([128, BC], F32, tag="ps_small", name="ps_cbc")
        nc.tensor.matmul(ps_cb[:], onesr_sb[:], c_row[:])
        c_bc = consts.tile([128, BC], F32, tag="cbc", name="c_bc")
        nc.scalar.activation(c_bc[:], ps_cb[:], ACTF.Copy)

        # accumulators
        ro_t = sm.tile([128, BC], F32, tag="rot", name="ro_t", bufs=1)
        ps_srow = ps_sm.tile([1, BC], F32, tag="ps_srow", name="ps_srow",
                             bufs=1)

        lt_tiles = [None] * BC
        g_state = [None] * BC

        def stage1(b):
            # pass 1: col-tiled logits + fix-up -> lt (PE + copies)
            mb = m_tiles[b]
            lt = ltp.tile([128, 1024], F32, tag="lt", name=f"lt{b}")
            lt_tiles[b] = lt
            for r in range(2):
                ps = ps_ct.tile([128, 512], F32, tag="ps_ct", name=f"psct{b}_{r}")
                for j in range(4):
                    nc.tensor.matmul(
                        ps[32 * j:32 * j + 32, :], qp32[:],
                        mb[:, (4 * r + j) * 512:(4 * r + j + 1) * 512],
                        start=True, stop=True, tile_position=(0, 32 * j))
                ct = ctp.tile([128, 512], F32, tag="ct", name=f"ct{b}_{r}")
                nc.any.tensor_copy(ct[:], ps[:])
                psf = ps_fx.tile([128, 512], F32, tag="ps_fx", name=f"psfx{b}_{r}")
                for q4 in range(4):
                    nc.tensor.matmul(psf[:, q4 * 128:(q4 + 1) * 128],
                                     ct[:, q4 * 128:(q4 + 1) * 128],
                                     ident_sb[:], is_transpose=True)
                nc.any.tensor_copy(
                    lt[:, r * 512:(r + 1) * 512].rearrange(
                        "p (j q2 q) -> p q2 j q", j=4, q2=4, q=32),
                    psf[:].rearrange("p (q2 j q) -> p q2 j q", q2=4, j=4, q=32))

        def stage2(b):
            # softmax chain -> g, fold to g_row
            lt = lt_tiles[b]
            wl = lt[:, b::32]
            lr = lt[:, (8 + b)::32]

            e1 = sm.tile([128, 32], F32, tag="e1", name=f"e1_{b}")
            e1s = sm.tile([128, 1], F32, tag="e1s", name=f"e1s_{b}")
            nc.scalar.activation(e1[:], wl, ACTF.Exp, accum_out=e1s[:])
            ps_s1 = ps_sm.tile([1, 1], F32, tag="ps_small", name=f"ps_s1_{b}")
            nc.tensor.matmul(ps_s1[:], e1s[:], onesc_sb[:, 0:1])
            s1 = sm.tile([1, 1], F32, tag="s1", name=f"s1_{b}")
            nc.any.tensor_copy(s1[:], ps_s1[:])
            r1 = sm.tile([1, 1], F32, tag="r1", name=f"r1_{b}")
            nc.vector.reciprocal(r1[:], s1[:])
            ps_r1 = ps_sm.tile([128, 1], F32, tag="ps_small", name=f"ps_r1_{b}")
            nc.tensor.matmul(ps_r1[:], onesr_sb[:], r1[:])
            r1c = sm.tile([128, 1], F32, tag="r1c", name=f"r1c_{b}")
            nc.any.tensor_copy(r1c[:], ps_r1[:])
            ww = sm.tile([128, 32], F32, tag="ww", name=f"ww_{b}")
            nc.vector.tensor_scalar_mul(ww[:], e1[:], r1c[:])

            t1 = sm.tile([128, 32], F32, tag="t1", name=f"t1_{b}")
            nc.vector.scalar_tensor_tensor(
                t1[:], lr, c_bc[:, b:b + 1], ww[:],
                op0=ALU.subtract, op1=ALU.mult)
            rl = sm.tile([128, 32], F32, tag="rl", name=f"rl_{b}")
            nc.vector.scalar_tensor_tensor(
                rl[:], lr, 0.0, t1[:], op0=ALU.add, op1=ALU.subtract)

            e2 = sm.tile([128, 32], F32, tag="e2", name=f"e2_{b}")
            e2s = sm.tile([128, 1], F32, tag="e2s", name=f"e2s_{b}")
            nc.scalar.activation(e2[:], rl[:], ACTF.Exp, accum_out=e2s[:])
            ps_s2 = ps_sm.tile([1, 1], F32, tag="ps_small", name=f"ps_s2_{b}")
            nc.tensor.matmul(ps_s2[:], e2s[:], onesc_sb[:, 0:1])
            s2 = sm.tile([1, 1], F32, tag="s2", name=f"s2_{b}")
            nc.any.tensor_copy(s2[:], ps_s2[:])
            r2 = sm.tile([1, 1], F32, tag="r2", name=f"r2_{b}")
            nc.vector.reciprocal(r2[:], s2[:])
            ps_r2 = ps_sm.tile([128, 1], F32, tag="ps_small", name=f"ps_r2_{b}")
            nc.tensor.matmul(ps_r2[:], onesr_sb[:], r2[:])
            r2c = sm.tile([128, 1], F32, tag="r2c", name=f"r2c_{b}")
            nc.any.tensor_copy(r2c[:], ps_r2[:])
            rw = sm.tile([128, 32], F32, tag="rw", name=f"rw_{b}")
            nc.vector.tensor_scalar_mul(rw[:], e2[:], r2c[:])

            t2 = sm.tile([128, 32], F32, tag="t2", name=f"t2_{b}")
            nc.vector.tensor_tensor(t2[:], rw[:], ww[:], ALU.mult)
            g = sm.tile([128, 32], F32, tag="g", name=f"g_{b}")
            nc.vector.tensor_tensor(g[:], rw[:], t2[:], ALU.subtract)
            t2s = sm.tile([128, 1], F32, tag="t2s", name=f"t2s_{b}")
            nc.vector.tensor_reduce(t2s[:], t2[:], AX.X, ALU.add)
            nc.tensor.matmul(ps_srow[0:1, b:b + 1], t2s[:], onesc_sb[:, 0:1])

            # n = 128*k' + nsub with k' = 16r + 4j + Q = g's compact col.
            ps_gt = ps_sm.tile([32, 128], F32, tag="ps_small",
                               name=f"ps_gt_{b}")
            nc.tensor.matmul(ps_gt[:], g[:], ident_sb[:], is_transpose=True)
            gt = sm.tile([32, 128], F32, tag="gt", name=f"gt_{b}")
            nc.any.tensor_copy(gt[:], ps_gt[:])
            g_row = grp.tile([1, N_SLOTS], gdt, tag="grow", name=f"grow_{b}",
                             bufs=2)
            nc.gpsimd.dma_start(g_row[0:1, :], gt[:])
            g_state[b] = g_row

        def stage3(b):
            # pass 2: per 512-quarter GB bcast (PE) + stt (DVE)
            mb = m_tiles[b]
            g_row = g_state[b]
            acc = sm.tile([128, 1], F32, tag="acc", name=f"acc_{b}")
            sttout = ctp.tile([128, 512], F32, tag="sttout", name=f"so_{b}")
            for qi in range(NGRP):
                psg = ps_gb.tile([128, 512], F32, tag="ps_gb",
                                 name=f"psgb{b}_{qi}")
                nc.tensor.matmul(psg[:], onesr_r[:],
                                 g_row[0:1, qi * 512:(qi + 1) * 512])
                acc_q = sm.tile([128, 1], F32, tag="accq",
                                name=f"accq_{b}_{qi}")
                nc.vector.scalar_tensor_tensor(
                    sttout[:], mb[:, qi * 512:(qi + 1) * 512], 1.0, psg[:],
                    op0=ALU.mult, op1=ALU.mult, accum_out=acc_q[:])
                if qi == 0:
                    nc.vector.tensor_copy(acc[:], acc_q[:])
                else:
                    nc.vector.tensor_tensor(acc[:], acc[:], acc_q[:], ALU.add)
            nc.vector.tensor_copy(ro_t[:, b:b + 1], acc[:])

        if phase == "p1":
            for b in range(BC):
                stage1(b)
            return
        if phase == "sm":
            for t in range(BC + 1):
                if t < BC:
                    stage1(t)
                if t >= 1:
                    stage2(t - 1)
            return
        for t in range(BC + 2):
            if t < BC:
                stage1(t)
            if t >= 2:
                stage3(t - 2)
            if t >= 1 and t - 1 < BC:
                stage2(t - 1)

        # ---------- epilogue ----------
        if phase != "full":
            return
        s_row = sm.tile([1, BC], F32, tag="srow", name="s_row")
        nc.any.tensor_copy(s_row[:], ps_srow[:])
        ps_sbc = ps_sm.tile([128, BC], F32, tag="ps_small", name="ps_sbc")
        nc.tensor.matmul(ps_sbc[:], onesr_sb[:], s_row[:])
        s_bc = sm.tile([128, BC], F32, tag="sbc", name="s_bc")
        nc.any.tensor_copy(s_bc[:], ps_sbc[:])

        t3 = sm.tile([128, BC], F32, tag="t3", name="t3")
        nc.vector.tensor_tensor(t3[:], mv_t[:], s_bc[:], ALU.mult)
        ro2 = sm.tile([128, BC], F32, tag="ro2", name="ro2")
        nc.vector.tensor_tensor(ro2[:], ro_t[:], t3[:], ALU.add)

        ps_out = ps_sm.tile([BC, IN_DIM], F32, tag="ps_small", name="ps_out")
        nc.tensor.matmul(ps_out[:], onesr_sb[:, 0:BC], b_ro_sb[:],
                         start=True, stop=False)
        nc.tensor.matmul(ps_out[:], ro2[:], w_ro_sb[:], start=False, stop=True)
        out_sb = sm.tile([BC, IN_DIM], F32, tag="outsb", name="out_sb")
        nc.any.tensor_copy(out_sb[:], ps_out[:])
        nc.sync.dma_start(out, out_sb[:])


_NC_CACHE = None


def _get_nc():
    global _NC_CACHE
    if _NC_CACHE is None:
        _NC_CACHE = build_nc()
    return _NC_CACHE


def make_in_maps(inputs):
    ident = np.eye(128, dtype=np.float32)
    ones_col = np.ones((128, 1), dtype=np.float32)
    ones_row = np.ones((1, 128), dtype=np.float32)
    shared = {
        "W_write": np.ascontiguousarray(inputs["W_write"], dtype=np.float32),
        "b_write": np.ascontiguousarray(inputs["b_write"], dtype=np.float32).reshape(1, D),
        "W_wq": np.ascontiguousarray(inputs["W_wq"], dtype=np.float32),
        "b_wq": np.ascontiguousarray(inputs["b_wq"], dtype=np.float32).reshape(1, D),
        "W_rq": np.ascontiguousarray(inputs["W_rq"], dtype=np.float32),
        "b_rq": np.ascontiguousarray(inputs["b_rq"], dtype=np.float32).reshape(1, D),
        "W_ro": np.ascontiguousarray(inputs["W_ro"], dtype=np.float32),
        "b_ro": np.ascontiguousarray(inputs["b_ro"], dtype=np.float32).reshape(1, IN_DIM),
        "ident": ident, "ones_col": ones_col, "ones_row": ones_row,
    }
    x = np.ascontiguousarray(inputs["x"], dtype=np.float32)
    memt = np.ascontiguousarray(
        np.asarray(inputs["memory"], dtype=np.float32).transpose(0, 2, 1))
    in_maps = []
    for i in range(N_CORES):
        m = dict(shared)
        m["x"] = np.ascontiguousarray(x[i * BC:(i + 1) * BC])
        m["memoryT"] = np.ascontiguousarray(memt[i * BC:(i + 1) * BC])
        in_maps.append(m)
    return in_maps


def kernel(**inputs) -> np.ndarray:
    nc = _get_nc()
    in_maps = make_in_maps(inputs)
    res = run_bass_kernel_spmd(nc, in_maps, list(range(N_CORES)))
    out = np.concatenate([res.results[i]["out"] for i in range(N_CORES)], axis=0)
    return np.ascontiguousarray(out, dtype=np.float32)


if __name__ == "__main__":
    nc = build_nc()
    print("built ok; instructions:",
          sum(len(bb.instructions) for bb in nc.main_func.blocks))

